# revision 25
# baseline (speedup 1.0000x reference)
"""Expert-parallel MoE SwiGLU FFN kernel for 8 Trainium2 NeuronCores.

Problem: T=4096 tokens, DIM=1024, E=8 experts, INTER=1408, top-2 routing.
Reference computes all experts densely then gathers; we instead route on the
host (sort token-slots by expert), assign one expert per core, and each core
runs a SwiGLU FFN over only its routed tokens (padded to a common capacity so
all 8 cores execute the same SPMD program).

Device layout (per core, everything "transposed" with tokens on the free dim):
  xt  [P,KT,C]   bf16   x_gathered.T tiled over DIM     (partition, k-tile, token)
  w1t [MT,P,1024] bf16  w1[e].T tiled over DIM (m-column-major)
  w3t [MT,P,1024] bf16
  w2t [MT,P,1024] bf16  w2[e].T tiled over INTER
  yt  [KT,P,C]   f32    y.T tiled over DIM (output)

Compute per core (per token-chunk of <=512):
  h1.T = w1 @ x.T   (accumulate over 8 DIM k-tiles)     -> PSUM [128, n]
  h3.T = w3 @ x.T
  g.T  = silu(h1.T) * h3.T                              -> SBUF bf16
  y.T  = w2 @ g.T   (accumulate over 11 INTER m-tiles)  -> PSUM -> SBUF f32 -> HBM

Performance notes (from perfetto traces):
  - The PE matmul stream is already at the bf16 streaming roofline
    (~264*C cycles @ 2.4 GHz); the remaining overheads are the head
    (DMA-trigger serialization: each dma_start costs ~640ns of sequencer
    time), the HAM cold-start (~3.4us at half clock), and the output tail.
  - So: few DMA triggers, issued in consumption order, spread over the three
    DMA-capable queues (sync/scalar HWDGE + gpsimd SWDGE); dummy warm-up
    matmuls run during the initial DMA window to flip the HAM clock gate to
    8/8 before the real stream starts; output DMAs alternate between two
    queues and the final tile is split in half so the drain tail is short.
"""

import numpy as np
import ml_dtypes

T, DIM, E, INTER, TOPK = 4096, 1024, 8, 1408, 2
NCORES = 8
P = 128
KT = DIM // P    # 8 k-tiles over DIM
MT = INTER // P  # 11 m-tiles over INTER
NWARM = 8        # warm-up matmuls to trip the HAM clock gate during DMA wait

TRACE = False  # test.py sets this to capture an NTFF profile
LAST_RESULTS = None  # BassKernelResults of the last run (for test.py)

_NC_CACHE = {}


def _chunks_for(C):
    # Split C into equal-ish chunks of at most 512 (PSUM bank = 512 fp32),
    # multiples of 16, avoiding a tiny LDWEIGHTS-bound tail chunk.
    nch = -(-C // 512)
    base = C // nch
    out = []
    rem = C
    for i in range(nch, 0, -1):
        n = min(512, -(-rem // i))
        n = -(-n // 16) * 16 if i > 1 else rem  # keep multiples of 16
        n = min(n, 512, rem)
        out.append(n)
        rem -= n
    assert sum(out) == C and all(0 < n <= 512 for n in out), out
    return out


def _build_nc(C):
    import concourse.mybir as mybir
    import concourse.tile as tile
    from concourse import bacc

    dt = mybir.dt
    AF = mybir.ActivationFunctionType
    chunks = _chunks_for(C)

    nc = bacc.Bacc(
        "TRN2", target_bir_lowering=False, debug=False, enable_asserts=False
    )
    # x is stored chunk-major (one contiguous [P, KT, n] block per token
    # chunk): the head of the kernel is HBM-supply-bound and chunk 0's
    # phase A only needs its own 0.7MB block, not all of x.
    xts = [
        nc.dram_tensor(f"xt{j}", [P, KT, n], dt.bfloat16, kind="ExternalInput")
        for j, n in enumerate(chunks)
    ]
    # m0..m3 of w1/w3 individually (latency-critical, consumed first); the
    # bulk as partition-major blocks so each DMA moves 6-8KB-contiguous
    # descriptor runs (2KB descriptors are ring-overhead-dominated).
    w1h = nc.dram_tensor("w1h", [4, P, KT * P], dt.bfloat16, kind="ExternalInput")
    w3h = nc.dram_tensor("w3h", [4, P, KT * P], dt.bfloat16, kind="ExternalInput")
    w1e = nc.dram_tensor("w1e", [P, 3, KT * P], dt.bfloat16, kind="ExternalInput")
    w3e = nc.dram_tensor("w3e", [P, 3, KT * P], dt.bfloat16, kind="ExternalInput")
    w1f = nc.dram_tensor("w1f", [P, 4, KT * P], dt.bfloat16, kind="ExternalInput")
    w3f = nc.dram_tensor("w3f", [P, 4, KT * P], dt.bfloat16, kind="ExternalInput")
    w2a = nc.dram_tensor("w2a", [P, 4, DIM], dt.bfloat16, kind="ExternalInput")
    w2b = nc.dram_tensor("w2b", [P, 4, DIM], dt.bfloat16, kind="ExternalInput")
    w2c = nc.dram_tensor("w2c", [P, 3, DIM], dt.bfloat16, kind="ExternalInput")
    yt = nc.dram_tensor("yt", [KT, P, C], dt.float32, kind="ExternalOutput")

    with tile.TileContext(nc) as tc:
        with (
            tc.tile_pool(name="persist", bufs=1) as wpool,
            tc.tile_pool(name="gbuf", bufs=3) as gpool,
            tc.tile_pool(name="ybuf", bufs=4) as ypool,
            tc.tile_pool(name="silbuf", bufs=3) as spool,
            tc.tile_pool(name="psA", bufs=3, space="PSUM") as psA,
            tc.tile_pool(name="psB", bufs=2, space="PSUM") as psB,
        ):
            # --- PE warm-up: dummy matmuls on a zeroed tile, emitted first so
            # they run during the initial DMA window and flip the HAM clock
            # gate (4/8 -> 8/8) before the real matmul stream begins.
            n0 = chunks[0]
            ws = wpool.tile([P, n0], dt.bfloat16, name="warm")
            nc.vector.memset(ws[:], 0)
            pw = psA.tile([P, n0], dt.float32, name="p1")
            for _ in range(NWARM):
                nc.tensor.matmul(pw[:], ws[:, :P], ws[:], start=True, stop=True)

            # SBUF layouts mirror the DRAM layouts so every DMA is contiguous
            # on both sides: per-chunk x tiles, m-major w1/w3/w2.
            xss = [wpool.tile([P, KT, n], dt.bfloat16, name=f"xs{j}")
                   for j, n in enumerate(chunks)]
            w1s = wpool.tile([P, MT, KT * P], dt.bfloat16)
            w3s = wpool.tile([P, MT, KT * P], dt.bfloat16)
            w2s = wpool.tile([P, MT, DIM], dt.bfloat16)

            # The input phase is HBM-bandwidth-bound (~0.4 GB/us aggregate,
            # all 16 SDMA rings saturated), and ring service is roughly FIFO
            # in enqueue order.  So enqueue strictly in consumption order,
            # alternating the two HWDGE trigger queues (sync + scalar, each
            # trigger costing ~640ns of serialized sequencer time) so
            # triggers retire pairwise.  x for chunks 1-2 and w2 are needed
            # tens of microseconds in, so they go last.  The gpsimd SWDGE is
            # far slower on input-sized transfers; it only carries outputs.
            nc.sync.dma_start(w1s[:, 0, :], w1h[0])
            nc.scalar.dma_start(xss[0][:, :KT // 2, :], xts[0][:, :KT // 2, :])
            nc.sync.dma_start(xss[0][:, KT // 2:, :], xts[0][:, KT // 2:, :])
            nc.scalar.dma_start(w3s[:, 0, :], w3h[0])
            nc.sync.dma_start(w1s[:, 1, :], w1h[1])
            nc.scalar.dma_start(w3s[:, 1, :], w3h[1])
            nc.sync.dma_start(w1s[:, 2, :], w1h[2])
            nc.scalar.dma_start(w3s[:, 2, :], w3h[2])
            nc.sync.dma_start(w1s[:, 3, :], w1h[3])
            nc.scalar.dma_start(w3s[:, 3, :], w3h[3])
            nc.sync.dma_start(w1s[:, 4:7, :], w1e[:])
            nc.scalar.dma_start(w3s[:, 4:7, :], w3e[:])
            nc.sync.dma_start(w1s[:, 7:, :], w1f[:])
            nc.scalar.dma_start(w3s[:, 7:, :], w3f[:])
            nc.sync.dma_start(w2s[:, :4, :], w2a[:])
            nc.scalar.dma_start(w2s[:, 4:8, :], w2b[:])
            nc.sync.dma_start(w2s[:, 8:, :], w2c[:])
            nc.scalar.dma_start(xss[1][:], xts[1][:])
            nc.sync.dma_start(xss[2][:], xts[2][:])

            out_engines = [nc.gpsimd, nc.sync]
            c0 = 0
            for j, n in enumerate(chunks):
                xsj = xss[j]
                gs = gpool.tile([P, MT, n], dt.bfloat16, name="gs")
                for m in range(MT):
                    p1 = psA.tile([P, n], dt.float32, name="p1")
                    p3 = psA.tile([P, n], dt.float32, name="p3")
                    for k in range(KT):
                        nc.tensor.matmul(
                            p1[:],
                            w1s[:, m, k * P:(k + 1) * P],
                            xsj[:, k, :],
                            start=(k == 0),
                            stop=(k == KT - 1),
                        )
                    for k in range(KT):
                        nc.tensor.matmul(
                            p3[:],
                            w3s[:, m, k * P:(k + 1) * P],
                            xsj[:, k, :],
                            start=(k == 0),
                            stop=(k == KT - 1),
                        )
                    sil = spool.tile([P, n], dt.bfloat16, name="sil")
                    nc.scalar.activation(sil[:], p1[:], AF.Silu)
                    nc.vector.tensor_mul(gs[:, m, :], sil[:], p3[:])
                for i in range(KT):
                    py = psB.tile([P, n], dt.float32, name="py")
                    for m in range(MT):
                        nc.tensor.matmul(
                            py[:],
                            w2s[:, m, i * P:(i + 1) * P],
                            gs[:, m, :],
                            start=(m == 0),
                            stop=(m == MT - 1),
                        )
                    ys = ypool.tile([P, n], dt.float32, name="ys")
                    last = (j == len(chunks) - 1) and (i == KT - 1)
                    if last:
                        # Split the final copy+store in half so the last DMA
                        # overlaps the copy and the drain tail is short.
                        h = (n // 2) // 4 * 4
                        nc.vector.tensor_copy(ys[:, :h], py[:, :h])
                        nc.scalar.dma_start(yt[i, :, c0:c0 + h], ys[:, :h])
                        nc.vector.tensor_copy(ys[:, h:], py[:, h:])
                        nc.sync.dma_start(yt[i, :, c0 + h:c0 + n], ys[:, h:])
                    else:
                        nc.vector.tensor_copy(ys[:], py[:])
                        eng = out_engines[(j * KT + i) % 2]
                        eng.dma_start(yt[i, :, c0:c0 + n], ys[:])
                c0 += n

    nc.compile()
    return nc


def _get_nc(C):
    if C not in _NC_CACHE:
        _NC_CACHE[C] = _build_nc(C)
    return _NC_CACHE[C]


def _ensure_ntff_hook_importable():
    # bass_utils imports antenv.axon_hooks when tracing is requested (e.g. via
    # a BASS_TRACE env var); in containers whose antenv stub lacks that
    # submodule the import would crash. Register a null hook so tracing just
    # degrades to "no trace" instead.
    import sys
    import types

    try:
        import antenv.axon_hooks  # noqa: F401
    except ImportError:
        mod = types.ModuleType("antenv.axon_hooks")
        mod.get_axon_ntff_profile_hook = lambda: None
        mod.set_axon_ntff_profile_hook = lambda hook: None
        sys.modules["antenv.axon_hooks"] = mod


def kernel(x, expert_indices, w1, w2, w3):
    global LAST_RESULTS
    import os
    import sys

    # The bass kernel executes on the NeuronCores via the axon PJRT backend;
    # a JAX_PLATFORMS=cpu pin (commonly used for running jax reference code)
    # would hide those devices. Clear it if jax hasn't initialized yet.
    if os.environ.get("JAX_PLATFORMS") == "cpu" and "jax" not in sys.modules:
        del os.environ["JAX_PLATFORMS"]

    from concourse import bass_utils

    _ensure_ntff_hook_importable()
    x = np.asarray(x, dtype=np.float32)
    idx = np.asarray(expert_indices)
    w1 = np.asarray(w1, dtype=np.float32)
    w2 = np.asarray(w2, dtype=np.float32)
    w3 = np.asarray(w3, dtype=np.float32)

    bf16 = ml_dtypes.bfloat16

    # --- host routing: stable-sort the (token, k) slots by expert id ---
    flat = idx.reshape(-1).astype(np.int64)  # slot s = t*TOPK + k -> expert
    order = np.argsort(flat, kind="stable")  # slots grouped by expert
    counts = np.bincount(flat, minlength=E)
    starts = np.zeros(E + 1, dtype=np.int64)
    np.cumsum(counts, out=starts[1:])
    cmax = int(counts.max())
    C = max(512, -(-cmax // 16) * 16)  # pad capacity to a multiple of 16

    nc = _get_nc(C)

    chunks = _chunks_for(C)
    bounds = np.cumsum([0] + chunks)
    xb = x.astype(bf16)
    in_maps = []
    for e in range(E):
        slots = order[starts[e]:starts[e + 1]]
        tokens = slots // TOPK
        xg = np.zeros((C, DIM), dtype=bf16)
        xg[: len(tokens)] = xb[tokens]
        # [C, DIM] -> [P, KT, C] (partition-major), then per-chunk blocks
        xpkc = xg.T.reshape(KT, P, C).transpose(1, 0, 2)
        im = {
            f"xt{j}": np.ascontiguousarray(xpkc[:, :, bounds[j]:bounds[j + 1]])
            for j in range(len(chunks))
        }
        # w1m[m, p, k*128+j] = w1[e][m*128+j, k*128+p]; first four m-tiles
        # stay m-major (latency-critical singles), the bulk goes
        # partition-major for long-contiguous DMA descriptors.
        w1m = w1[e].astype(bf16).reshape(MT, P, KT, P).transpose(0, 3, 2, 1)
        w1m = np.ascontiguousarray(w1m).reshape(MT, P, KT * P)
        w3m = w3[e].astype(bf16).reshape(MT, P, KT, P).transpose(0, 3, 2, 1)
        w3m = np.ascontiguousarray(w3m).reshape(MT, P, KT * P)
        w2m = np.ascontiguousarray(w2[e].T.astype(bf16)).reshape(MT, P, DIM)
        im["w1h"] = w1m[:4]
        im["w3h"] = w3m[:4]
        im["w1e"] = np.ascontiguousarray(w1m[4:7].transpose(1, 0, 2))
        im["w3e"] = np.ascontiguousarray(w3m[4:7].transpose(1, 0, 2))
        im["w1f"] = np.ascontiguousarray(w1m[7:].transpose(1, 0, 2))
        im["w3f"] = np.ascontiguousarray(w3m[7:].transpose(1, 0, 2))
        im["w2a"] = np.ascontiguousarray(w2m[:4].transpose(1, 0, 2))
        im["w2b"] = np.ascontiguousarray(w2m[4:8].transpose(1, 0, 2))
        im["w2c"] = np.ascontiguousarray(w2m[8:].transpose(1, 0, 2))
        in_maps.append(im)

    res = bass_utils.run_bass_kernel_spmd(
        nc, in_maps, core_ids=list(range(NCORES)), trace=TRACE
    )
    LAST_RESULTS = res

    out = np.empty((T * TOPK, DIM), dtype=np.float32)
    for e in range(E):
        slots = order[starts[e]:starts[e + 1]]
        yt = res.results[e]["yt"]  # [KT, P, C] f32
        y = yt.reshape(DIM, C)  # y.T
        out[slots] = y[:, : len(slots)].T
    return out.reshape(T, TOPK, DIM)
